# revision 32
# baseline (speedup 1.0000x reference)
"""DeepSeek MLA head — Trainium2 Bass kernel, 8 NeuronCores.

Sharding: 8 cores = 2 batches x 4 cores. Each core owns one batch and 4 of
the 16 heads (tensor-parallel over heads within a batch, data-parallel over
batch across core groups). The low-rank latent projections are sharded over
TOKENS within each 4-core group (each core computes latents for its 512
tokens) and AllGathered post-RMSNorm, removing the 4x-replicated latent
compute of the naive layout. Each core emits a partial o_proj output
[S, HID] (f16) which the host sums per batch.

Layout strategy: activations kept transposed [feature, token] on-chip so
every matmul contraction lands on the partition axis with no on-device
transposes. Host pre-transposes x, folds RMSNorm gains + the DeepSeek RoPE
interleave permutation into the weight matrices, and pads kv_a so the
shared k_pe lands on partition rows 64:128.

Numerics: all matmul operands f16 (f32 PSUM accumulation), softmax in f32
on the ScalarE with a fixed -4 bias (exp(SCALE*s - 4)); the bias cancels in
the normalization and keeps the f16 row-sum accumulators far from
overflow. Softmax denominators accumulate on the VectorE in f16 (tree of
two accumulators) instead of per-chunk PE ones-matmuls.
"""
import sys
import types

sys.path.insert(0, "/opt/trn_rl_repo")

import numpy as np

B, S, HID, NH = 2, 2048, 2048, 16
ROPE, NOPE, VDIM = 64, 64, 128
QHEAD, QLORA, KVLORA = 128, 682, 256
THETA = 128000.0
SCALE = 1.0 / float(np.sqrt(128.0))
EPS = 1e-6
EXPB = -4.0          # exp bias: exp(SCALE*s + EXPB), cancels in softmax
HPC = 4              # heads per core
NCORES = 8
QCH = [128, 128, 128, 128, 128, 42]   # qlora partition chunks
NST = 4              # 512-token supertiles per batch
STW = 512

_PROGRAM = None


def _ensure_axon_hooks_shim():
    if "antenv.axon_hooks" in sys.modules:
        return
    try:
        from trn_agent_boot.trn_boot import _ntff_profile_via_ctypes
        hook = _ntff_profile_via_ctypes("/opt/axon/libaxon_pjrt.so")
    except Exception:
        hook = None
    m = types.ModuleType("antenv.axon_hooks")
    m.get_axon_ntff_profile_hook = lambda: hook
    m.set_axon_ntff_profile_hook = lambda h: None
    sys.modules["antenv.axon_hooks"] = m


def _build_program():
    import concourse.bass as bass  # noqa: F401
    import concourse.mybir as mybir
    import concourse.tile as tile
    from concourse import bacc

    f16 = mybir.dt.float16
    f32 = mybir.dt.float32
    AF = mybir.ActivationFunctionType

    nc = bacc.Bacc("TRN2", target_bir_lowering=False, debug=False,
                   num_devices=NCORES)
    # register const APs so activation(bias=...) works
    eps_t = nc.alloc_sbuf_tensor("const-eps", [128, 1], f32)
    nc.gpsimd.memset(eps_t.ap(), EPS)
    nc.const_aps.aps[(f32, EPS)] = eps_t.ap()
    expb_t = nc.alloc_sbuf_tensor("const-expb", [128, 1], f32)
    nc.gpsimd.memset(expb_t.ap(), EXPB)
    nc.const_aps.aps[(f32, EXPB)] = expb_t.ap()
    nc.all_engine_barrier()

    def din(name, shape, dt=f16):
        return nc.dram_tensor(name, shape, dt, kind="ExternalInput").ap()

    xT = din("xT", [HID, STW])            # this core's 512-token slice of x
    waq = din("waq", [HID, QLORA])        # q_a_w
    wakv = din("wakv", [HID, 384])        # kv_a_w cols: [ckv 256 | 0s 64 | kpe-perm 64]
    wqb = din("wqb", [QLORA, HPC * 128])  # per head: [nope64 | pe64-perm], ln folded
    wkn = din("wkn", [KVLORA, HPC * 64])  # per head: knope cols, ln folded
    wv = din("wv", [KVLORA, HPC * 128])   # per head: v cols, ln folded
    wo = din("wo", [HPC * VDIM, HID])     # o_w rows for this core's heads
    cosT = din("cosT", [64, S])           # cos rows for the rope half
    sinT = din("sinT", [64, S])
    cosL = din("cosL", [64, STW])         # local-token slice of cosT
    sinL = din("sinL", [64, STW])
    rotT = din("rotT", [64, 64])          # transposed rotate-half (rope rows)
    maskT = din("maskT", [128, 4 * STW])  # causal diagonal masks j=0..3
    out = nc.dram_tensor("out", [S, HID], f16, kind="ExternalOutput").ap()

    RG = [[0, 1, 2, 3], [4, 5, 6, 7]]     # token/head groups share a batch
    KVROWS = KVLORA + ROPE                # 320 rows in the kv AG payload

    from contextlib import ExitStack
    with tile.TileContext(nc) as tc, ExitStack() as ctx:
        const = ctx.enter_context(tc.tile_pool(name="const", bufs=1))
        waqp = ctx.enter_context(tc.tile_pool(name="waqp", bufs=16))
        wakvp = ctx.enter_context(tc.tile_pool(name="wakvp", bufs=16))
        xtp = ctx.enter_context(tc.tile_pool(name="xtp", bufs=16))
        latqp = ctx.enter_context(tc.tile_pool(name="latqp", bufs=6))
        latkp = ctx.enter_context(tc.tile_pool(name="latkp", bufs=1))
        rawp = ctx.enter_context(tc.tile_pool(name="rawp", bufs=8))
        sqp = ctx.enter_context(tc.tile_pool(name="sqp", bufs=2))
        tmpp = ctx.enter_context(tc.tile_pool(name="tmpp", bufs=3))
        smallp = ctx.enter_context(tc.tile_pool(name="smallp", bufs=2))
        bcp = ctx.enter_context(tc.tile_pool(name="bcp", bufs=1))
        persist = ctx.enter_context(tc.tile_pool(name="persist", bufs=HPC))
        vtp = ctx.enter_context(tc.tile_pool(name="vtp", bufs=1))
        ptp = ctx.enter_context(tc.tile_pool(name="ptp", bufs=4))
        accp = ctx.enter_context(tc.tile_pool(name="accp", bufs=4))
        oep = ctx.enter_context(tc.tile_pool(name="oep", bufs=2))
        dram = ctx.enter_context(tc.tile_pool(name="dram", bufs=1, space="DRAM"))
        ps_a = ctx.enter_context(tc.tile_pool(name="ps_a", bufs=3, space="PSUM"))
        ps_b = ctx.enter_context(tc.tile_pool(name="ps_b", bufs=2, space="PSUM"))
        ps_c = ctx.enter_context(tc.tile_pool(name="ps_c", bufs=1, space="PSUM"))
        ps_e = ctx.enter_context(tc.tile_pool(name="ps_e", bufs=2, space="PSUM"))

        # ---- constants into SBUF ----
        sb_waq = [waqp.tile([128, QLORA], f16, tag="waq", name=f"waq{hc}")
                  for hc in range(16)]
        sb_wakv = [wakvp.tile([128, 384], f16, tag="wakv", name=f"wakv{hc}")
                   for hc in range(16)]
        sb_wqb = const.tile([128, 6 * HPC * 128], f16, tag="wqb")
        sb_wkn = const.tile([128, 2 * HPC * 64], f16, tag="wkn")
        sb_wv = const.tile([128, 2 * HPC * 128], f16, tag="wv")
        sb_cos = const.tile([64, S], f16, tag="cos")
        sb_sin = const.tile([64, S], f16, tag="sin")
        sb_cosl = const.tile([64, STW], f16, tag="cosl")
        sb_sinl = const.tile([64, STW], f16, tag="sinl")
        sb_rot = const.tile([64, 64], f16, tag="rot")
        sb_mask = const.tile([128, 4 * STW], f16, tag="mask")
        sb_ones = const.tile([128, 1], f16, tag="ones")
        sb_onesr = const.tile([1, 128], f16, tag="onesr")

        qoff = [0, 128, 256, 384, 512, 640]
        W = HPC * 128
        # DMA priority: x + wakv gate the kv latents -> AG1; everything else
        # is ordered by first use so the startup critical path stays short.
        xt = [xtp.tile([128, STW], f16, tag="xt", name=f"xt{hc}")
              for hc in range(16)]
        for hc in range(16):
            nc.sync.dma_start(out=xt[hc][:], in_=xT[hc * 128:(hc + 1) * 128, :])
            nc.sync.dma_start(out=sb_wakv[hc][:], in_=wakv[hc * 128:(hc + 1) * 128, :])
        nc.sync.dma_start(out=sb_cosl[:], in_=cosL[:])
        nc.sync.dma_start(out=sb_sinl[:], in_=sinL[:])
        nc.sync.dma_start(out=sb_rot[:], in_=rotT[:])
        for hc in range(16):
            nc.sync.dma_start(out=sb_waq[hc][:], in_=waq[hc * 128:(hc + 1) * 128, :])
        for c in range(2):
            nc.sync.dma_start(out=sb_wkn[:, c * HPC * 64:(c + 1) * HPC * 64],
                              in_=wkn[c * 128:(c + 1) * 128, :])
            nc.sync.dma_start(out=sb_wv[:, c * W:(c + 1) * W],
                              in_=wv[c * 128:(c + 1) * 128, :])
        nc.vector.memset(sb_ones[:], 1.0)
        nc.vector.memset(sb_onesr[:], 1.0)
        warm = smallp.tile([1, 1], f16, tag="warm", bufs=1)
        nc.vector.memset(warm[:], 0.0)
        nc.scalar.activation(warm[:], warm[:], AF.Exp, bias=EXPB, scale=SCALE)

        # AG bounce buffers (HBM)
        ag1_in = dram.tile([KVROWS, STW], f16, tag="ag1i")
        ag1_out = dram.tile([4 * KVROWS, STW], f16, tag="ag1o")
        ag2_in = dram.tile([QLORA, STW], f16, tag="ag2i")
        ag2_out = dram.tile([4 * QLORA, STW], f16, tag="ag2o")

        # persistent per-head tensors
        qfT = [persist.tile([128, S], f16, tag="qf", name=f"qfT{h}") for h in range(HPC)]
        kfT = [persist.tile([128, S], f16, tag="kf", name=f"kfT{h}") for h in range(HPC)]
        VT = vtp.tile([128, 16 * HPC * VDIM], f16, tag="vt", name="VT")
        aout = [persist.tile([128, S], f16, tag="aout", name=f"aout{h}") for h in range(HPC)]
        # gathered latents [feature, all tokens]
        latq = [latqp.tile([128, S], f16, tag="latq", name=f"latq{c}") for c in range(6)]
        latk = [latkp.tile([128, S], f16, tag=f"latk{c}", name=f"latk{c}") for c in range(2)]
        latpe = latkp.tile([64, S], f16, tag="latpe", name="latpe")

        # ================= P1a: local latents (512 tokens) =================
        # -- kv first so AG1 can fly while q latents compute --
        kraws = []
        bck = ps_c.tile([128, STW], f32, tag="bc", name="bck")
        ksums = bck[0:1, :]
        for c in range(2):
            ps = ps_a.tile([128, STW], f32, tag="big")
            for hc in range(16):
                nc.tensor.matmul(ps[:], sb_wakv[hc][:, c * 128:(c + 1) * 128],
                                 xt[hc][:], start=(hc == 0), stop=(hc == 15))
            raw = rawp.tile([128, STW], f16, tag="raw")
            nc.scalar.copy(out=raw[:], in_=ps[:])
            sq = sqp.tile([128, STW], f16, tag="sq")
            nc.vector.tensor_mul(sq[:], raw[:], raw[:])
            nc.tensor.matmul(ksums[:], sb_ones[:, :], sq[:],
                             start=(c == 0), stop=(c == 1))
            kraws.append(raw)
        ps = ps_a.tile([128, STW], f32, tag="big")
        for hc in range(16):
            nc.tensor.matmul(ps[:], sb_wakv[hc][:, 256:384], xt[hc][:],
                             start=(hc == 0), stop=(hc == 15))
        kperaw = tmpp.tile([64, STW], f16, tag="kpe64", name="kperaw", bufs=1)
        nc.scalar.copy(out=kperaw[:], in_=ps[64:128, :])

        stdkf = smallp.tile([1, STW], f32, tag="stdf", name="stdkf")
        nc.scalar.activation(stdkf[:], ksums[:], AF.Sqrt, bias=EPS,
                             scale=1.0 / KVLORA)
        rstdkf = smallp.tile([1, STW], f32, tag="stdf", name="rstdkf")
        nc.vector.reciprocal_approx_fast(out=rstdkf[:], in_=stdkf[:])
        rstdk = smallp.tile([1, STW], f16, tag="std")
        nc.vector.tensor_copy(rstdk[:], rstdkf[:])
        nc.tensor.matmul(bck[:], sb_onesr[:], rstdk[:1, :], start=True, stop=True)
        bcks = bcp.tile([128, STW], f16, tag="bc")
        nc.scalar.copy(out=bcks[:], in_=bck[:])
        for c in range(2):
            nc.vector.tensor_mul(kraws[c][:], kraws[c][:], bcks[:])

        # shared k_pe rope on local tokens (rows 64:128 of kperaw)
        rps = ps_a.tile([128, STW], f32, tag="big")
        nc.tensor.matmul(rps[0:64, :], sb_rot[:], kperaw[:], start=True, stop=True)
        t1 = tmpp.tile([128, STW], f16, tag="tmp", name="t1")
        nc.vector.tensor_mul(t1[0:64, :], rps[0:64, :], sb_sinl[:])
        t2 = tmpp.tile([128, STW], f16, tag="tmp", name="t2")
        nc.vector.tensor_mul(t2[0:64, :], kperaw[:], sb_cosl[:])
        kpero = tmpp.tile([64, STW], f16, tag="kpero", name="kpero", bufs=1)
        nc.vector.tensor_add(kpero[:], t1[0:64, :], t2[0:64, :])

        # AG1: [kraw0 | kraw1 | kpe]; reloads ride the sync queue so the
        # gpsimd queue is free to trigger AG2 the moment AG1 completes.
        for c in range(2):
            nc.gpsimd.dma_start(ag1_in[c * 128:(c + 1) * 128, :], kraws[c][:])
        nc.gpsimd.dma_start(ag1_in[256:320, :], kpero[:])
        nc.gpsimd.collective_compute(
            "AllGather", mybir.AluOpType.bypass, replica_groups=RG,
            ins=[ag1_in.opt()], outs=[ag1_out.opt()])
        for r in range(4):
            for c in range(2):
                nc.sync.dma_start(
                    out=latk[c][:, r * STW:(r + 1) * STW],
                    in_=ag1_out[r * KVROWS + c * 128:r * KVROWS + (c + 1) * 128, :])
            nc.sync.dma_start(
                out=latpe[:, r * STW:(r + 1) * STW],
                in_=ag1_out[r * KVROWS + 256:r * KVROWS + 320, :])

        # weights/tables needed from the q_b stage onward
        for c in range(6):
            nc.sync.dma_start(out=sb_wqb[:QCH[c], c * W:(c + 1) * W],
                              in_=wqb[qoff[c]:qoff[c] + QCH[c], :])
        nc.sync.dma_start(out=sb_cos[:], in_=cosT[:])
        nc.sync.dma_start(out=sb_sin[:], in_=sinT[:])
        nc.sync.dma_start(out=sb_mask[:], in_=maskT[:])

        # -- q latents --
        qraws = []
        bcq = ps_c.tile([128, STW], f32, tag="bc", name="bcq")
        qsums = bcq[0:1, :]
        for c in range(6):
            ps = ps_a.tile([128, STW], f32, tag="big")
            for hc in range(16):
                nc.tensor.matmul(ps[:QCH[c], :],
                                 sb_waq[hc][:, qoff[c]:qoff[c] + QCH[c]],
                                 xt[hc][:], start=(hc == 0), stop=(hc == 15))
            raw = rawp.tile([128, STW], f16, tag="raw")
            nc.scalar.copy(out=raw[:QCH[c], :], in_=ps[:QCH[c], :])
            sq = sqp.tile([128, STW], f16, tag="sq")
            nc.vector.tensor_mul(sq[:QCH[c], :], raw[:QCH[c], :], raw[:QCH[c], :])
            nc.tensor.matmul(qsums[:], sb_ones[:QCH[c], :], sq[:QCH[c], :],
                             start=(c == 0), stop=(c == 5))
            qraws.append(raw)
        stdqf = smallp.tile([1, STW], f32, tag="stdf", name="stdqf")
        nc.scalar.activation(stdqf[:], qsums[:], AF.Sqrt, bias=EPS,
                             scale=1.0 / QLORA)
        rstdqf = smallp.tile([1, STW], f32, tag="stdf", name="rstdqf")
        nc.vector.reciprocal_approx_fast(out=rstdqf[:], in_=stdqf[:])
        rstdq = smallp.tile([1, STW], f16, tag="std")
        nc.vector.tensor_copy(rstdq[:], rstdqf[:])
        nc.tensor.matmul(bcq[:], sb_onesr[:], rstdq[:1, :], start=True, stop=True)
        bcqs = bcp.tile([128, STW], f16, tag="bc")
        nc.scalar.copy(out=bcqs[:], in_=bcq[:])
        for c in range(6):
            nc.vector.tensor_mul(qraws[c][:QCH[c], :], qraws[c][:QCH[c], :],
                                 bcqs[:QCH[c], :])
            nc.sync.dma_start(out=ag2_in[qoff[c]:qoff[c] + QCH[c], :],
                              in_=qraws[c][:QCH[c], :])
        nc.gpsimd.collective_compute(
            "AllGather", mybir.AluOpType.bypass, replica_groups=RG,
            ins=[ag2_in.opt()], outs=[ag2_out.opt()])
        for r in range(4):        # first-consumed supertile reloads first
            for c in range(6):
                nc.sync.dma_start(
                    out=latq[c][:QCH[c], r * STW:(r + 1) * STW],
                    in_=ag2_out[r * QLORA + qoff[c]:r * QLORA + qoff[c] + QCH[c], :])

        # ================= P1b: K/V per head over all tokens ===============
        for st in range(NST):
            cols = slice(st * STW, (st + 1) * STW)
            # k_nope: head pairs (2 heads x 64 rows per matmul)
            for pr in range(2):
                psk = ps_a.tile([128, STW], f32, tag="big")
                for c in range(2):
                    nc.tensor.matmul(
                        psk[:],
                        sb_wkn[:, c * 256 + pr * 128:c * 256 + (pr + 1) * 128],
                        latk[c][:, cols], start=(c == 0), stop=(c == 1))
                nc.scalar.copy(out=kfT[2 * pr][0:64, cols], in_=psk[0:64, :])
                nc.vector.tensor_copy(kfT[2 * pr + 1][0:64, cols], psk[64:128, :])
            for h in range(HPC):
                nc.vector.tensor_copy(kfT[h][64:128, cols], latpe[:, cols])
            # V: all 4 heads at once per 128-token chunk
            for tcn in range(4):
                kc = st * 4 + tcn
                psv = ps_a.tile([128, STW], f32, tag="big")
                for c in range(2):
                    nc.tensor.matmul(
                        psv[:],
                        latk[c][:, st * STW + tcn * 128:st * STW + (tcn + 1) * 128],
                        sb_wv[:, c * W:(c + 1) * W],
                        start=(c == 0), stop=(c == 1))
                if tcn % 2 == 0:
                    nc.vector.tensor_copy(VT[:, kc * W:(kc + 1) * W], psv[:])
                else:
                    nc.scalar.copy(out=VT[:, kc * W:(kc + 1) * W], in_=psv[:])

        # o_w loads into the xt stream slots (xt dead after the q latents)
        sb_wo = [xtp.tile([128, STW], f16, tag="xt", name=f"wo{c}_{hcn}")
                 for c in range(HPC) for hcn in range(4)]
        for c in range(HPC):
            for hcn in range(4):
                nc.sync.dma_start(
                    out=sb_wo[c * 4 + hcn][:],
                    in_=wo[c * 128:(c + 1) * 128, hcn * STW:(hcn + 1) * STW])

        # ================= P1c + P2 + P3 per query supertile ===============
        for st in range(NST):
            cols = slice(st * STW, (st + 1) * STW)
            # --- q_b + rope; the rot matmul trails one head so the PE never
            #     waits on the scalar psq->qpe copy ---
            qpes = [None] * HPC

            def _emit_qb(h, cols=cols):
                psq = ps_a.tile([128, STW], f32, tag="big")
                for c in range(6):
                    nc.tensor.matmul(
                        psq[:],
                        sb_wqb[:QCH[c], c * W + h * 128:c * W + (h + 1) * 128],
                        latq[c][:QCH[c], cols], start=(c == 0), stop=(c == 5))
                nc.scalar.copy(out=qfT[h][0:64, cols], in_=psq[0:64, :])
                qpe = tmpp.tile([64, STW], f16, tag="qpe64", name=f"qpe{h}", bufs=2)
                nc.scalar.copy(out=qpe[:], in_=psq[64:128, :])
                qpes[h] = qpe

            def _emit_rope(h, cols=cols):
                qpe = qpes[h]
                rq = ps_a.tile([128, STW], f32, tag="big")
                nc.tensor.matmul(rq[0:64, :], sb_rot[:], qpe[:], start=True, stop=True)
                t1q = tmpp.tile([128, STW], f16, tag="tmp", name="t1q")
                nc.vector.tensor_mul(t1q[0:64, :], rq[0:64, :], sb_sin[:, cols])
                t2q = tmpp.tile([128, STW], f16, tag="tmp", name="t2q")
                nc.vector.tensor_mul(t2q[0:64, :], qpe[:], sb_cos[:, cols])
                nc.vector.tensor_add(qfT[h][64:128, cols], t1q[0:64, :], t2q[0:64, :])

            _emit_qb(0)
            for h in range(1, HPC):
                _emit_qb(h)
                _emit_rope(h - 1)
            _emit_rope(HPC - 1)

            # --- attention; PV trails QK by 2 chunks and the previous
            #     head's softmax tail is woven into this head's stream so
            #     the PE never waits on the ScalarE exp or the VectorE
            #     reductions ---
            qs = st
            qcols = slice(qs * STW, (qs + 1) * STW)
            nkc = 4 * qs + 4
            prev = None      # (h, outT, accs, bca, rs) awaiting tail parts

            def _nt_sums(state):
                ph, poutT, paccs = state
                live = [a for a in paccs if a is not None]
                bca = ps_c.tile([128, STW], f32, tag="bc", name=f"bca{ph}")
                for i, a in enumerate(live):
                    nc.tensor.matmul(bca[0:1, :], sb_ones[:, :], a[:],
                                     start=(i == 0), stop=(i == len(live) - 1))
                rsf = smallp.tile([1, STW], f32, tag="stdf", name="rsf")
                nc.vector.reciprocal_approx_fast(out=rsf[:], in_=bca[0:1, :])
                rs = smallp.tile([1, STW], f16, tag="std", name="rs")
                nc.vector.tensor_copy(rs[:], rsf[:])
                return (ph, poutT, paccs, bca, rs)

            def _nt_rest(state, qcols=qcols):
                ph, poutT, paccs, bca, rs = state
                nc.tensor.matmul(bca[:], sb_onesr[:], rs[:1, :],
                                 start=True, stop=True)
                bcas = bcp.tile([128, STW], f16, tag="bc")
                nc.scalar.copy(out=bcas[:], in_=bca[:])
                nc.vector.tensor_mul(aout[ph][:, qcols], poutT[:], bcas[:])

            for h in range(HPC):
                outT = ps_b.tile([128, STW], f32, tag="hold")
                acc = [None, None]
                pvq = []
                for kc in range(nkc):
                    j = kc - 4 * qs
                    v0 = 128 * j if j > 0 else 0   # first valid q col
                    stp = ps_a.tile([128, STW], f32, tag="big")
                    nc.tensor.matmul(stp[:, v0:STW],
                                     kfT[h][:, kc * 128:(kc + 1) * 128],
                                     qfT[h][:, qs * STW + v0:(qs + 1) * STW],
                                     start=True, stop=True)
                    pt = ptp.tile([128, STW], f16, tag="pt")
                    if v0 > 0:
                        nc.gpsimd.memset(pt[:, 0:v0], 0.0)
                    nc.scalar.activation(pt[:, v0:STW], stp[:, v0:STW], AF.Exp,
                                         bias=EXPB, scale=SCALE)
                    if j >= 0:
                        nc.vector.tensor_mul(
                            pt[:, v0:STW], pt[:, v0:STW],
                            sb_mask[:, j * STW + v0:(j + 1) * STW])
                    a = kc % 2
                    if acc[a] is None:
                        acc[a] = accp.tile([128, STW], f16, tag="acc",
                                           name=f"acc{h}_{a}")
                        nc.vector.tensor_copy(acc[a][:], pt[:])
                    else:
                        nc.vector.tensor_add(acc[a][:], acc[a][:], pt[:])
                    pvq.append((kc, v0, pt))
                    if len(pvq) > 2:
                        k0, w0, p0 = pvq.pop(0)
                        nc.tensor.matmul(
                            outT[:, w0:STW],
                            VT[:, k0 * W + h * 128:k0 * W + (h + 1) * 128],
                            p0[:, w0:STW], start=(k0 == 0), stop=False)
                    if prev is not None and kc == 1:
                        prev = _nt_sums(prev)
                    elif prev is not None and kc == 3:
                        _nt_rest(prev)
                        prev = None
                for k0, w0, p0 in pvq:
                    nc.tensor.matmul(
                        outT[:, w0:STW],
                        VT[:, k0 * W + h * 128:k0 * W + (h + 1) * 128],
                        p0[:, w0:STW], start=(k0 == 0), stop=(k0 == nkc - 1))
                prev = (h, outT, acc)
            _nt_rest(_nt_sums(prev))
            prev = None

            # --- o projection for this supertile's tokens ---
            for tcn in range(4 * st, 4 * st + 4):
                for hcn in range(4):
                    pso = ps_e.tile([128, STW], f32, tag="oe")
                    for h in range(HPC):
                        nc.tensor.matmul(
                            pso[:], aout[h][:, tcn * 128:(tcn + 1) * 128],
                            sb_wo[h * 4 + hcn][:],
                            start=(h == 0), stop=(h == HPC - 1))
                    ob = oep.tile([128, STW], f16, tag="oe")
                    if (tcn + hcn) % 2 == 0:
                        nc.vector.tensor_copy(ob[:], pso[:])
                    else:
                        nc.scalar.copy(out=ob[:], in_=pso[:])
                    nc.sync.dma_start(
                        out=out[tcn * 128:(tcn + 1) * 128, hcn * STW:(hcn + 1) * STW],
                        in_=ob[:])

    nc.compile()
    return nc


def _host_prep(inputs):
    f16 = np.float16
    x = np.asarray(inputs["x"], np.float32)
    q_a_w = np.asarray(inputs["q_a_w"], np.float32)
    q_a_ln = np.asarray(inputs["q_a_ln_w"], np.float32)
    q_b_w = np.asarray(inputs["q_b_w"], np.float32)
    kv_a_w = np.asarray(inputs["kv_a_w"], np.float32)
    kv_a_ln = np.asarray(inputs["kv_a_ln_w"], np.float32)
    kv_b_w = np.asarray(inputs["kv_b_w"], np.float32)
    o_w = np.asarray(inputs["o_w"], np.float32)

    perm = np.concatenate([np.arange(0, ROPE, 2), np.arange(1, ROPE, 2)])
    q_b_f = q_b_w * q_a_ln[:, None]
    kv_b_f = kv_b_w * kv_a_ln[:, None]

    # kv_a padded: [ckv 256 | zeros 64 | kpe perm 64]
    wakv = np.concatenate(
        [kv_a_w[:, :KVLORA],
         np.zeros((HID, 64), np.float32),
         kv_a_w[:, KVLORA:][:, perm]], axis=1).astype(f16)
    waq = q_a_w.astype(f16)

    # rope tables (transposed [dim, pos])
    inv = 1.0 / (THETA ** (np.arange(0, ROPE, 2, dtype=np.float64) / ROPE))
    freqs = np.outer(np.arange(S, dtype=np.float64), inv)      # [S, 32]
    cos64 = np.concatenate([np.cos(freqs), np.cos(freqs)], -1).T  # [64, S]
    sin64 = np.concatenate([np.sin(freqs), np.sin(freqs)], -1).T
    cosT = cos64.astype(f16)
    sinT = sin64.astype(f16)

    # rotate-half matrix: out = ROT @ xp, nonzero only on rows/cols 64:128
    R64 = np.zeros((64, 64), np.float32)
    for j in range(32):
        R64[j, 32 + j] = -1.0
        R64[32 + j, j] = 1.0
    # rotT[k, m]: rot_half(x)[m] = sum_k rotT[k, m] * x[k] on the rope half
    rotT = R64.T.astype(np.float32).astype(f16)

    # diagonal causal masks: mask_j[k, q] = k <= q - 128*j
    k_i = np.arange(128)[:, None]
    q_i = np.arange(STW)[None, :]
    maskT = np.concatenate(
        [(k_i <= q_i - 128 * j).astype(f16) for j in range(4)], axis=1)

    in_maps = []
    for core in range(NCORES):
        b = core // 4
        r = core % 4
        heads = [HPC * (core % 4) + i for i in range(HPC)]
        tcols = slice(r * STW, (r + 1) * STW)
        wqb = np.concatenate(
            [np.concatenate(
                [q_b_f[:, h * QHEAD:h * QHEAD + NOPE],
                 q_b_f[:, h * QHEAD + NOPE:(h + 1) * QHEAD][:, perm]], 1)
             for h in heads], axis=1).astype(f16)
        wkn = np.concatenate(
            [kv_b_f[:, h * (NOPE + VDIM):h * (NOPE + VDIM) + NOPE]
             for h in heads], axis=1).astype(f16)
        wv = np.concatenate(
            [kv_b_f[:, h * (NOPE + VDIM) + NOPE:(h + 1) * (NOPE + VDIM)]
             for h in heads], axis=1).astype(f16)
        wo = np.concatenate(
            [o_w[h * VDIM:(h + 1) * VDIM, :] for h in heads], axis=0).astype(f16)
        in_maps.append({
            "xT": np.ascontiguousarray(x[b, tcols, :].T).astype(f16),
            "waq": waq, "wakv": wakv, "wqb": wqb, "wkn": wkn, "wv": wv,
            "wo": wo, "cosT": cosT, "sinT": sinT,
            "cosL": np.ascontiguousarray(cosT[:, tcols]),
            "sinL": np.ascontiguousarray(sinT[:, tcols]),
            "rotT": rotT, "maskT": maskT,
        })
    return in_maps


def kernel(**inputs):
    global _PROGRAM
    _ensure_axon_hooks_shim()
    from concourse.bass_utils import run_bass_kernel_spmd

    if _PROGRAM is None:
        _PROGRAM = _build_program()
    in_maps = _host_prep(inputs)
    res = run_bass_kernel_spmd(_PROGRAM, in_maps, list(range(NCORES)))
    out = np.zeros((B, S, HID), np.float32)
    for core in range(NCORES):
        out[core // 4] += res.results[core]["out"].astype(np.float32)
    return out
